# revision 12
# baseline (speedup 1.0000x reference)
"""Per-pixel adaptive 5x5 conv (KPN apply) on 8 Trainium2 NeuronCores.

out[b,c,h,w] = sum_{i,j} core[b,0,i*5+j,c,h,w] * frames[b,0,c,h+i-2,w+j-2]
(zero-padded borders), output [4,3,512,512] f32.

Sharding: pure data parallel, core k -> (b = k//2, H-half = k%2).

v2: all device inputs are bfloat16 (truncated f32 high halves, taken as
zero-copy numpy views on the host so the only host-side gather happens
once, inside the runner). Halves every byte moved: host memcpy, host->
device transfer, and HBM traffic on-core. Kernel math: products and the
accumulate chain in bf16 on DVE (2x packed mode), final add emits f32.
Measured rel err of this scheme vs the f32 reference: ~9e-3 (gate 2e-2).

Raw-bass implementation (the walrus build in this env only allows one
semaphore wait per compute/DMA instruction, so Tile's auto-sync can't be
used): explicit double-buffered pipeline, standalone waits, all loads/
stores on the SP HWDGE FIFO so ordering among DMAs is implicit.

Per 128-row block: five DMAs bring the 25 tap planes [128, 5, 512] bf16
each (chunked so DVE can start after the first chunk lands), two DMAs
bring 5-row overlapping windows of the padded frame (ftA, and ftB
shifted one column so odd-j tap slices stay 4-byte aligned for the DVE
2x packed mode). DVE writes 25 products into a [128, 25, 512] tile,
then reduces with a 6-op pairwise tree over contiguous multi-tap slices
(large free dims amortize the per-op fixed cost; all slices step-1 and
4B-aligned so the 2x packed mode holds); the last add writes f32.
"""

import numpy as np
import ml_dtypes

import concourse.bass as bass
import concourse.mybir as mybir
from concourse.ap import AP
from concourse.bass_utils import run_bass_kernel_spmd

B, N, C, H, W = 4, 1, 3, 512, 512
K = 5
PAD = K // 2
NCORES = 8
HH = H // (NCORES // B)  # 256 rows per core
P = 128
NBLK_TOT = C * (HH // P)  # 6 blocks of 128 rows per core
WPAD = W + 2 * PAD        # 516
BF16 = ml_dtypes.bfloat16

_CACHE = {}


def _build():
    nc = bass.Bass()
    f32 = mybir.dt.float32
    bf16 = mybir.dt.bfloat16

    fr = nc.declare_dram_parameter("fr", [C, HH + 2 * PAD, WPAD], bf16, isOutput=False)
    co = nc.declare_dram_parameter("co", [K * K, C, HH, W], bf16, isOutput=False)
    out = nc.declare_dram_parameter("out", [C, HH, W], f32, isOutput=True)

    def co_view(n, g):
        # tap chunk g (5 taps) of block n, as [P, 5, W]
        c, blk = n // (HH // P), n % (HH // P)
        return co[5 * g:5 * g + 5, c, blk * P:blk * P + P, :].transpose([1, 0, 2])

    def fr_win(n, shift):
        # [P, K, 516-or-515] window of the padded frame, rows overlapping;
        # shift=1 reads one column later (for odd-j taps).
        c, blk = n // (HH // P), n % (HH // P)
        fb = fr[c, blk * P:blk * P + P, :]
        return AP(fb.tensor, fb.offset + shift,
                  [(WPAD, P), (WPAD, K), (1, WPAD - shift)])

    def out_view(n):
        c, blk = n // (HH // P), n % (HH // P)
        return out[c, blk * P:blk * P + P, :]

    with (
        nc.sbuf_tensor("ct0", [P, K * K, W], bf16) as ct0,
        nc.sbuf_tensor("ct1", [P, K * K, W], bf16) as ct1,
        nc.sbuf_tensor("fa0", [P, K, WPAD], bf16) as fa0,
        nc.sbuf_tensor("fa1", [P, K, WPAD], bf16) as fa1,
        nc.sbuf_tensor("fb0", [P, K, WPAD], bf16) as fb0,
        nc.sbuf_tensor("fb1", [P, K, WPAD], bf16) as fb1,
        nc.sbuf_tensor("prd", [P, K * K, W], bf16) as prd,
        nc.sbuf_tensor("oa0", [P, W], f32) as oa0,
        nc.sbuf_tensor("oa1", [P, W], f32) as oa1,
        # Only the LAST DMA of a guarded set increments its semaphore: the
        # per-engine FIFO order of the SP HWDGE ring then guarantees every
        # earlier DMA's descriptors have drained when the count arrives
        # (cumulative counts across DMAs would race: a fast engine can
        # deliver a later DMA's increment while a slow engine still owes
        # one from an earlier DMA).
        nc.semaphore("csem") as csem,   # fta,ftb,ct0-2 done (ct2 incs +16)
        nc.semaphore("dsem") as dsem,   # whole block loaded (ct4 incs +16)
        nc.semaphore("osem") as osem,   # store completions (+16 per DMA)
        nc.semaphore("vsem") as vsem,   # DVE per-block completion (+1)
        nc.semaphore("xsem") as xsem,   # bookkeeping only (never waited on;
                                        # this walrus build requires sync
                                        # info on every dynamic DMA)
        nc.Block() as block,
    ):
        cts, fas, fbs = [ct0, ct1], [fa0, fa1], [fb0, fb1]
        oas = [oa0, oa1]
        NG = K  # 5 tap chunks per block

        @block.sync
        def _(sync: bass.BassEngine):
            for n in range(NBLK_TOT):
                if n >= 2:
                    # DVE done with block n-2 => its buffers reusable and
                    # its f32 acc ready to store.
                    sync.wait_ge(vsem, n - 1)
                    sync.dma_start(
                        out=out_view(n - 2), in_=oas[n % 2][:]
                    ).then_inc(osem, 16)
                sync.dma_start(out=fas[n % 2][:], in_=fr_win(n, 0)).then_inc(xsem, 16)
                sync.dma_start(
                    out=fbs[n % 2][:, :, 0:WPAD - 1], in_=fr_win(n, 1)
                ).then_inc(xsem, 16)
                for g in range(NG):
                    ins = sync.dma_start(
                        out=cts[n % 2][:, 5 * g:5 * g + 5, :], in_=co_view(n, g)
                    )
                    if g == 2:
                        ins.then_inc(csem, 16)
                    elif g == NG - 1:
                        ins.then_inc(dsem, 16)
                    else:
                        ins.then_inc(xsem, 16)
            sync.wait_ge(vsem, NBLK_TOT - 1)
            sync.dma_start(
                out=out_view(NBLK_TOT - 2), in_=oas[NBLK_TOT % 2][:]
            ).then_inc(osem, 16)
            sync.wait_ge(vsem, NBLK_TOT)
            sync.dma_start(
                out=out_view(NBLK_TOT - 1), in_=oas[(NBLK_TOT + 1) % 2][:]
            ).then_inc(osem, 16)
            sync.wait_ge(osem, 16 * NBLK_TOT)

        @block.vector
        def _(vector: bass.BassEngine):
            for n in range(NBLK_TOT):
                ct, fta, ftb = cts[n % 2], fas[n % 2], fbs[n % 2]
                oac = oas[n % 2]
                for t in range(K * K):
                    i, j = t // K, t % K
                    if t == 0:
                        # frames + ct chunks 0-2 of this block landed
                        vector.wait_ge(csem, 16 * (n + 1))
                    elif t == 15:
                        # rest of the block landed
                        vector.wait_ge(dsem, 16 * (n + 1))
                    csl = ct[:, t, :]
                    # odd j reads the shifted copy so the slice stays
                    # 4B-aligned (DVE 2x packed mode requirement)
                    fsl = fta[:, i, j:j + W] if j % 2 == 0 \
                        else ftb[:, i, j - 1:j - 1 + W]
                    vector.tensor_mul(out=prd[:, t, :], in0=csl, in1=fsl)
                # pairwise tree over contiguous tap slices: 25 = (8+8+8)+1
                vector.tensor_add(
                    out=prd[:, 0:8, :], in0=prd[:, 0:8, :], in1=prd[:, 8:16, :])
                vector.tensor_add(
                    out=prd[:, 0:8, :], in0=prd[:, 0:8, :], in1=prd[:, 16:24, :])
                vector.tensor_add(
                    out=prd[:, 0:4, :], in0=prd[:, 0:4, :], in1=prd[:, 4:8, :])
                vector.tensor_add(
                    out=prd[:, 0:2, :], in0=prd[:, 0:2, :], in1=prd[:, 2:4, :])
                vector.tensor_add(
                    out=prd[:, 0, :], in0=prd[:, 0, :], in1=prd[:, 1, :])
                if n >= 2:
                    # store of block n-2 (same f32 acc buffer) must be done
                    vector.wait_ge(osem, 16 * (n - 1))
                vector.tensor_add(
                    out=oac[:], in0=prd[:, 0, :], in1=prd[:, 24, :]
                ).then_inc(vsem, 1)
    return nc


def get_nc():
    if "nc" not in _CACHE:
        _CACHE["nc"] = _build()
    return _CACHE["nc"]


def _as_bf16_trunc(a):
    # Zero-copy bf16 view: the high 16 bits of each f32 (little-endian).
    # Truncation (not round-to-nearest); max rel err 2^-8 per element.
    return a.view(np.uint16)[..., 1::2].view(BF16)


def shard_inputs(frames, core):
    frames = np.asarray(frames)
    core = np.asarray(core)
    if frames.dtype != np.float32:
        frames = frames.astype(np.float32)
    if core.dtype != np.float32:
        core = core.astype(np.float32)
    if not frames.flags.c_contiguous:
        frames = np.ascontiguousarray(frames)
    if not core.flags.c_contiguous:
        core = np.ascontiguousarray(core)
    fr_bf = _as_bf16_trunc(frames)  # [B,1,C,H,W] bf16 view
    co_bf = _as_bf16_trunc(core)    # [B,1,25,C,H,W] bf16 view
    # One small padded copy per batch (the halo rows / W padding); all
    # per-core entries below are views, so the only large gather happens
    # once, inside the runner (concat / tobytes).
    fp = np.empty((B, C, H + 2 * PAD, WPAD), BF16)
    fp[:, :, :PAD, :] = 0
    fp[:, :, PAD + H:, :] = 0
    fp[:, :, :, :PAD] = 0
    fp[:, :, :, PAD + W:] = 0
    fp[:, :, PAD:PAD + H, PAD:PAD + W] = fr_bf[:, 0]
    in_maps = []
    for k in range(NCORES):
        b, half = k // 2, k % 2
        h0 = half * HH
        in_maps.append({
            "fr": fp[b, :, h0:h0 + HH + 2 * PAD, :],
            "co": co_bf[b, 0, :, :, h0:h0 + HH, :],
        })
    return in_maps


def run(in_maps, **kwargs):
    return run_bass_kernel_spmd(get_nc(), in_maps, list(range(NCORES)), **kwargs)


def kernel(frames, core):
    in_maps = shard_inputs(frames, core)
    res = run(in_maps).results
    outp = np.empty((B, C, H, W), np.float32)
    for k in range(NCORES):
        b, half = k // 2, k % 2
        outp[b, :, half * HH:(half + 1) * HH, :] = res[k]["out"]
    return outp
